# revision 15
# baseline (speedup 1.0000x reference)
"""AttentionPairBias (Pairformer) 8-core sequence-parallel Bass kernel.

Shapes (hardcoded): B=1, L=768, c_a=384, c_pair=128, H=16, c=24.
Sharding: i-axis split 8 ways -> LS=96 rows of i per core; weights + A replicated.

Per-core dataflow (all on one NeuronCore, no collectives):
  phase A: LN(A) -> a^T via PE transpose; k^T/q^T/v/g via PE matmuls
           (LN affine + 1/sqrt(c) folded into weights host-side; q/k head dims
           padded 24->32 so per-head slices never straddle partition tiles)
  phase Z: per i-row, DMA Z^T[i] (d on partitions), square it (DVE/ACT),
           one PE matmul vs [gamma*Wb | ones] over [z | z^2] gives the 16 head
           projections + sum(z) + sum(z^2) for 4 i-rows packed into PSUM via
           column tiling; copy PSUM->SBUF, DMA-permute into [i, j] channel tiles
  phase B: LN stats from s1/s2 channels; per head: bias = rstd*(proj - c1*mu)
           (+beta), logits = QK^T + I@bias in PSUM, Exp with accum_out (softmax
           denominator for free), P^T via PE transpose, P@V, gate*recip, out@Wa
"""

import sys

for _p in ("/opt/trn_rl_repo", "/root/.axon_site/_ro/trn_rl_repo"):
    if _p not in sys.path:
        sys.path.append(_p)

import numpy as np
import ml_dtypes

L = 768
NCORES = 8
LS = L // NCORES  # 96
CA = 384
CP = 128
H = 16
C = 24
EP = 512  # padded q/k feature dim (16 heads x 32)
LN_EPS = 1e-5
BF = ml_dtypes.bfloat16

# tuning knobs
ZBUFS = 6          # Z tile double-buffering depth
SQ_ACT_MOD = 5     # i % SQ_ACT_MOD < SQ_ACT_CNT -> square on ScalarE else VectorE
SQ_ACT_CNT = 2
STRIP_ACT_MOD = 2  # alternate strip copies between ScalarE/VectorE


def build_body(nc, tc, ins, out_ap, ls):
    """Emit the tile program. ins: dict name->AP (DRAM), out_ap: DRAM AP [ls, 384]."""
    import concourse.bass as bass
    from concourse import mybir
    from concourse.masks import make_identity

    fp32 = mybir.dt.float32
    bf16 = mybir.dt.bfloat16
    AF = mybir.ActivationFunctionType
    OP = mybir.AluOpType
    ngrp = ls // 4

    consts = tc.alloc_tile_pool(name="consts", bufs=1)
    persist = tc.alloc_tile_pool(name="persist", bufs=1)

    # ---- constants ----
    id96 = consts.tile([96, 96], bf16, tag="id96", name="id96")
    make_identity(nc, id96)
    id128 = consts.tile([128, 128], bf16, tag="id128", name="id128")
    make_identity(nc, id128)
    ones_row = consts.tile([1, L], bf16, tag="ones_row", name="ones_row")
    nc.vector.memset(ones_row, 1.0)
    eps128 = consts.tile([128, 1], fp32, tag="eps128", name="eps128")
    nc.vector.memset(eps128, LN_EPS)
    eps96 = consts.tile([ls, 1], fp32, tag="eps96", name="eps96")
    nc.vector.memset(eps96, LN_EPS)

    def bcast96(name, width, tag):
        t = consts.tile([ls, width], fp32, tag=tag, name=tag)
        src = ins[name]
        nc.sync.dma_start(
            out=t, in_=bass.AP(tensor=src.tensor, offset=src.offset,
                               ap=[[0, ls]] + list(src.ap[1:]))
        )
        return t

    c1nbc = bcast96("c1n", H, "c1nbc")  # [96, 16] = -sum_d(g0*Wb)
    c2bc = bcast96("c2", H, "c2bc")     # [96, 16] = ln0_b @ Wb

    # ---- weights to SBUF ----
    def load_w(name, cols, nchunk, tag):
        ts = []
        for k in range(nchunk):
            t = consts.tile([128, cols], bf16, tag=f"{tag}{k}", name=f"{tag}{k}")
            nc.sync.dma_start(out=t, in_=ins[name][128 * k:128 * (k + 1), :])
            ts.append(t)
        return ts

    wq_sb = load_w("wq", EP, 3, "wq")
    wk_sb = load_w("wk", EP, 3, "wk")
    wv_sb = load_w("wv", CA, 3, "wv")
    wg_sb = load_w("wg", CA, 3, "wg")
    wa_sb = load_w("wa", CA, 3, "wa")
    waug = consts.tile([128, 32], bf16, tag="waug", name="waug")
    nc.sync.dma_start(out=waug, in_=ins["waug"])
    brow = {}
    for nm, w in (("bq", EP), ("bk", EP), ("bv", CA), ("bg", CA)):
        brow[nm] = consts.tile([1, w], bf16, tag=nm, name=nm)
        nc.sync.dma_start(out=brow[nm], in_=ins[nm])

    # ---- persistent activations ----
    aT = [persist.tile([128, L], bf16, tag=f"aT{k}", name=f"aT{k}") for k in range(3)]
    aTi = [persist.tile([128, ls], bf16, tag=f"aTi{k}", name=f"aTi{k}") for k in range(3)]
    kT = [persist.tile([128, L], bf16, tag=f"kT{m}", name=f"kT{m}") for m in range(4)]
    qT = [persist.tile([128, ls], bf16, tag=f"qT{m}", name=f"qT{m}") for m in range(4)]
    v_sb = [persist.tile([128, CA], bf16, tag=f"v{j}", name=f"v{j}") for j in range(6)]
    g_sb = persist.tile([ls, CA], bf16, tag="g", name="g")
    chan = persist.tile([ls, 18, L], bf16, tag="chan", name="chan")
    o_all = persist.tile([ls, CA], bf16, tag="o_all", name="o_all")

    # ================= phase A =================
    with tc.tile_pool(name="apool", bufs=3) as apool, \
         tc.tile_pool(name="aps_big", bufs=2, space="PSUM") as aps_big, \
         tc.tile_pool(name="aps_sm", bufs=2, space="PSUM") as aps_sm:

        def layernorm(dst, src_dram, rows, cols, epst):
            x = apool.tile([rows, cols], fp32, tag="ln_x", name="ln_x")
            nc.sync.dma_start(out=x, in_=src_dram)
            st = apool.tile([rows, 6], fp32, tag="ln_st", name="ln_st")
            nc.vector.bn_stats(out=st, in_=x)
            mv = apool.tile([rows, 2], fp32, tag="ln_mv", name="ln_mv")
            nc.vector.bn_aggr(out=mv, in_=st)
            nc.scalar.activation(out=mv[:, 1:2], in_=mv[:, 1:2], func=AF.Sqrt,
                                 bias=epst[:rows])
            nc.vector.reciprocal(out=mv[:, 1:2], in_=mv[:, 1:2])
            nc.vector.tensor_scalar(out=dst, in0=x, scalar1=mv[:, 0:1],
                                    scalar2=mv[:, 1:2],
                                    op0=OP.subtract, op1=OP.mult)

        # full-A LN + transpose -> aT
        for t in range(6):
            a_ln = apool.tile([128, CA], bf16, tag="a_ln", name="a_ln")
            layernorm(a_ln, ins["a_full"][128 * t:128 * (t + 1), :], 128, CA, eps128)
            for k in range(3):
                ps = aps_sm.tile([128, 128], bf16, tag="tr", name="tr")
                nc.tensor.transpose(ps, a_ln[:, 128 * k:128 * (k + 1)], id128)
                nc.scalar.copy(out=aT[k][:, 128 * t:128 * (t + 1)], in_=ps)

        # core-slice LN + transpose -> aTi
        ai_ln = apool.tile([ls, CA], bf16, tag="ai_ln", name="ai_ln")
        layernorm(ai_ln, ins["a_i"], ls, CA, eps96)
        for k in range(3):
            ps = aps_sm.tile([128, 128], bf16, tag="tr", name="tr")
            nc.tensor.transpose(ps[:, :ls], ai_ln[:, 128 * k:128 * (k + 1)], id96[:ls, :ls])
            nc.scalar.copy(out=aTi[k], in_=ps[:, :ls])

        # kT[m] = (Wk'^T @ a^T)[128 cols of e', all j] + bias
        for m in range(4):
            ps = aps_big.tile([128, L], fp32, tag="kq", name="kq")
            for nch in range(2):
                cs = slice(512 * nch, min(512 * (nch + 1), L))
                for k in range(3):
                    nc.tensor.matmul(ps[:, cs], wk_sb[k][:, 128 * m:128 * (m + 1)],
                                     aT[k][:, cs], start=(k == 0), stop=False)
                nc.tensor.matmul(ps[:, cs], brow["bk"][:, 128 * m:128 * (m + 1)],
                                 ones_row[:, cs], start=False, stop=True)
            nc.vector.tensor_copy(out=kT[m], in_=ps)

        # qT[m] (only this core's i-slice)
        for m in range(4):
            ps = aps_sm.tile([128, 128], fp32, tag="qps", name="qps")
            for k in range(3):
                nc.tensor.matmul(ps[:, :ls], wq_sb[k][:, 128 * m:128 * (m + 1)],
                                 aTi[k], start=(k == 0), stop=False)
            nc.tensor.matmul(ps[:, :ls], brow["bq"][:, 128 * m:128 * (m + 1)],
                             ones_row[:, :ls], start=False, stop=True)
            nc.scalar.copy(out=qT[m], in_=ps[:, :ls])

        # v[j] = a @ Wv' + bv   (natural layout, j on partitions)
        for j in range(6):
            ps = aps_big.tile([128, L], fp32, tag="kq", name="kq")
            for k in range(3):
                nc.tensor.matmul(ps[:, :CA], aT[k][:, 128 * j:128 * (j + 1)],
                                 wv_sb[k], start=(k == 0), stop=False)
            nc.tensor.matmul(ps[:, :CA], ones_row[:, :128],
                             brow["bv"], start=False, stop=True)
            nc.vector.tensor_copy(out=v_sb[j], in_=ps[:, :CA])

        # g = sigmoid(a_i @ Wg' + bg)
        ps = aps_big.tile([128, L], fp32, tag="kq", name="kq")
        for k in range(3):
            nc.tensor.matmul(ps[:ls, :CA], aTi[k], wg_sb[k],
                             start=(k == 0), stop=False)
        nc.tensor.matmul(ps[:ls, :CA], ones_row[:, :ls], brow["bg"],
                         start=False, stop=True)
        nc.scalar.activation(out=g_sb, in_=ps[:ls, :CA], func=AF.Sigmoid)

    # ================= phase Z =================
    with tc.tile_pool(name="zpool", bufs=ZBUFS) as zpool, \
         tc.tile_pool(name="strips", bufs=3) as strips, \
         tc.tile_pool(name="zps", bufs=2, space="PSUM") as zps:
        ps_g = None
        for i in range(ls):
            gi, c = i % 4, i // 4
            zt = zpool.tile([128, 2 * L], bf16, tag="z", name="z")
            nc.sync.dma_start(out=zt[:, 0:L], in_=ins["zt"][i])
            if i % SQ_ACT_MOD < SQ_ACT_CNT:
                nc.scalar.square(out=zt[:, L:2 * L], in_=zt[:, 0:L])
            else:
                nc.vector.tensor_tensor(out=zt[:, L:2 * L], in0=zt[:, 0:L],
                                        in1=zt[:, 0:L], op=mybir.AluOpType.mult)
            if gi == 0:
                ps_g = zps.tile([128, 2 * L], fp32, tag="zp", name="zp")
            for nch in range(3):
                cs = slice(512 * nch, 512 * (nch + 1))
                nc.tensor.matmul(ps_g[32 * gi:32 * gi + 32, cs], waug, zt[:, cs],
                                 start=True, stop=True, tile_position=(0, 32 * gi))
            if gi == 3:
                strip = strips.tile([128, 2 * L], bf16, tag="strip", name="strip")
                if c % STRIP_ACT_MOD == 0:
                    nc.scalar.copy(out=strip, in_=ps_g)
                else:
                    nc.vector.tensor_copy(out=strip, in_=ps_g)
                for gj in range(4):
                    nc.sync.dma_start(
                        out=chan[4 * c + gj:4 * c + gj + 1, 0:17, :],
                        in_=strip[32 * gj:32 * gj + 17, 0:L])
                    nc.sync.dma_start(
                        out=chan[4 * c + gj:4 * c + gj + 1, 17:18, :],
                        in_=strip[32 * gj + 16:32 * gj + 17, L:2 * L])

    # ================= phase B =================
    from concourse import mybir as _mb
    with tc.tile_pool(name="bpool", bufs=3) as bpool, \
         tc.tile_pool(name="bstat", bufs=1) as bstat, \
         tc.tile_pool(name="bps_l", bufs=2, space="PSUM") as bps_l, \
         tc.tile_pool(name="bps_pta", bufs=1, space="PSUM") as bps_pta, \
         tc.tile_pool(name="bps_ptb", bufs=1, space="PSUM") as bps_ptb, \
         tc.tile_pool(name="bps_o", bufs=2, space="PSUM") as bps_o:

        # stats: mu, rstd  (per position, [ls, L])
        mu = bstat.tile([ls, L], bf16, tag="mu", name="mu")
        nc.vector.tensor_scalar_mul(out=mu, in0=chan[:, 16, :], scalar1=1.0 / CP)
        m2 = bstat.tile([ls, L], fp32, tag="m2", name="m2")
        nc.vector.tensor_tensor(out=m2, in0=mu, in1=mu, op=OP.mult)
        var = bstat.tile([ls, L], fp32, tag="var", name="var")
        nc.vector.scalar_tensor_tensor(out=var, in0=chan[:, 17, :],
                                       scalar=1.0 / CP, in1=m2,
                                       op0=OP.mult, op1=OP.subtract)
        nc.scalar.activation(out=var, in_=var, func=AF.Sqrt, bias=eps96)
        rstd32 = bstat.tile([ls, L], fp32, tag="rstd32", name="rstd32")
        nc.vector.reciprocal_approx_fast(out=rstd32, in_=var)
        rstd = bstat.tile([ls, L], bf16, tag="rstd", name="rstd")
        nc.vector.tensor_copy(out=rstd, in_=rstd32)
        betaF = bstat.tile([ls, L], bf16, tag="betaF", name="betaF")
        nc.sync.dma_start(out=betaF, in_=ins["beta"])

        for h in range(H):
            # bias_h = rstd * (proj_h - c1_h * mu) + beta
            t_h = bpool.tile([ls, L], bf16, tag="t_h", name="t_h")
            nc.vector.scalar_tensor_tensor(out=t_h, in0=mu,
                                           scalar=c1nbc[:, h:h + 1],
                                           in1=chan[:, h, :],
                                           op0=OP.mult, op1=OP.add)
            b_h = bpool.tile([ls, L], bf16, tag="b_h", name="b_h")
            nc.vector.scalar_tensor_tensor(out=b_h, in0=t_h, scalar=0.0,
                                           in1=rstd, op0=OP.add, op1=OP.mult)
            bias_h = bpool.tile([ls, L], bf16, tag="bias_h", name="bias_h")
            nc.vector.tensor_tensor(out=bias_h, in0=b_h, in1=betaF, op=OP.add)

            # logits = q_h^T k_h + bias_h   (PSUM accumulate)
            ps_l = bps_l.tile([ls, L], fp32, tag="lg", name="lg")
            m, r = h // 4, 32 * (h % 4)
            for nch in range(2):
                cs = slice(512 * nch, min(512 * (nch + 1), L))
                nc.tensor.matmul(ps_l[:, cs], qT[m][r:r + 32, :],
                                 kT[m][r:r + 32, cs], start=True, stop=False,
                                 tile_position=(r, 0))
                nc.tensor.matmul(ps_l[:, cs], id96[:ls, :ls], bias_h[:, cs],
                                 start=False, stop=True)

            # P = exp(logits + c2_h), denom = row-sum(P)
            P_h = bpool.tile([ls, L], bf16, tag="P", name="P")
            den = bpool.tile([ls, 1], fp32, tag="den", name="den")
            nc.scalar.activation(out=P_h, in_=ps_l, func=AF.Exp,
                                 bias=c2bc[:, h:h + 1], accum_out=den)

            # P^T via PE transpose (6 chunks of [96,128] -> [128,96])
            pta = bps_pta.tile([128, 384], bf16, tag="pta", name="pta")
            ptb = bps_ptb.tile([128, 384], bf16, tag="ptb", name="ptb")
            for c6 in range(6):
                dst = pta if c6 < 3 else ptb
                col = 128 * (c6 % 3)
                nc.tensor.transpose(dst[:, col:col + ls],
                                    P_h[:, 128 * c6:128 * (c6 + 1)], id96[:ls, :ls])
            PT = bpool.tile([128, 6 * ls], bf16, tag="PT", name="PT")
            for half, psrc in ((0, pta), (1, ptb)):
                nc.scalar.copy(
                    out=PT[:, 3 * ls * half:3 * ls * (half + 1)].rearrange(
                        "p (c w) -> p c w", c=3),
                    in_=psrc.rearrange("p (c w) -> p c w", c=3)[:, :, 0:ls])

            # O_h = P V   -> gate & normalize
            ps_o = bps_o.tile([ls, CA], fp32, tag="o", name="o")
            for c6 in range(6):
                nc.tensor.matmul(ps_o[:, 0:C], PT[:, ls * c6:ls * (c6 + 1)],
                                 v_sb[c6][:, C * h:C * (h + 1)],
                                 start=(c6 == 0), stop=(c6 == 5))
            rden = bpool.tile([ls, 1], fp32, tag="rden", name="rden")
            nc.vector.reciprocal(out=rden, in_=den)
            nc.vector.scalar_tensor_tensor(out=o_all[:, C * h:C * (h + 1)],
                                           in0=ps_o[:, 0:C], scalar=rden,
                                           in1=g_sb[:, C * h:C * (h + 1)],
                                           op0=OP.mult, op1=OP.mult)

        # out = (g * O) @ Wa
        ot_ps = bps_pta.tile([128, 384], bf16, tag="pta", name="pta")
        for k in range(3):
            nc.tensor.transpose(ot_ps[:, 128 * k:128 * k + ls],
                                o_all[:, 128 * k:128 * (k + 1)], id96[:ls, :ls])
        OT = bpool.tile([128, 3 * ls], bf16, tag="OT", name="OT")
        nc.scalar.copy(out=OT.rearrange("p (c w) -> p c w", c=3),
                       in_=ot_ps.rearrange("p (c w) -> p c w", c=3)[:, :, 0:ls])
        ps_out = bps_o.tile([ls, CA], fp32, tag="o", name="o")
        for k in range(3):
            nc.tensor.matmul(ps_out, OT[:, ls * k:ls * (k + 1)], wa_sb[k],
                             start=(k == 0), stop=(k == 2))
        out_sb = bpool.tile([ls, CA], fp32, tag="out_sb", name="out_sb")
        nc.vector.tensor_copy(out=out_sb, in_=ps_out)
        nc.sync.dma_start(out=out_ap, in_=out_sb)

    persist.release()
    consts.release()


def build_program(ls=LS):
    from concourse import bacc, mybir
    import concourse.tile as tile

    bf16 = mybir.dt.bfloat16
    fp32 = mybir.dt.float32
    nc = bacc.Bacc()
    specs = {
        "zt": ([ls, CP, L], bf16),
        "beta": ([ls, L], bf16),
        "a_full": ([L, CA], fp32),
        "a_i": ([ls, CA], fp32),
        "wq": ([CA, EP], bf16),
        "wk": ([CA, EP], bf16),
        "wv": ([CA, CA], bf16),
        "wg": ([CA, CA], bf16),
        "wa": ([CA, CA], bf16),
        "waug": ([CP, 32], bf16),
        "bq": ([1, EP], bf16),
        "bk": ([1, EP], bf16),
        "bv": ([1, CA], bf16),
        "bg": ([1, CA], bf16),
        "c1n": ([1, H], fp32),
        "c2": ([1, H], fp32),
    }
    ins = {}
    for name, (shape, dt) in specs.items():
        h = nc.dram_tensor(name, shape, dt, kind="ExternalInput")
        ins[name] = h[tuple(slice(None) for _ in shape)]
    out_h = nc.dram_tensor("out", [ls, CA], fp32, kind="ExternalOutput")
    out_ap = out_h[:, :]
    with tile.TileContext(nc) as tc:
        build_body(nc, tc, ins, out_ap, ls)
    nc.finalize()
    return nc


def host_prep(A_I, Z_II, Beta_II, Wq, Wk, Wv, Wg, Wb, Wa, ln0_g, ln0_b,
              ln1_g, ln1_b, ls=LS, ncores=NCORES):
    """Fold LN affines into weights, shard+transpose Z. Returns in_maps list."""
    s = 1.0 / np.sqrt(C)
    g1 = np.asarray(ln1_g, np.float32)[:, None]
    b1 = np.asarray(ln1_b, np.float32)

    def fold(w, scale, pad):
        wf = np.asarray(w, np.float32).reshape(CA, CA)
        wfold = (g1 * wf) * scale
        brow = (b1 @ wf) * scale
        if pad:
            wp = np.zeros((CA, EP), np.float32)
            bp = np.zeros((1, EP), np.float32)
            for h in range(H):
                wp[:, 32 * h:32 * h + C] = wfold[:, C * h:C * (h + 1)]
                bp[0, 32 * h:32 * h + C] = brow[C * h:C * (h + 1)]
            return wp.astype(BF), bp.astype(BF)
        return wfold.astype(BF), brow[None, :].astype(BF)

    wq, bq = fold(Wq, s, True)
    wk, bk = fold(Wk, 1.0, True)
    wv, bv = fold(Wv, 1.0, False)
    wg, bg = fold(Wg, 1.0, False)
    wb = np.asarray(Wb, np.float32)
    g0 = np.asarray(ln0_g, np.float32)[:, None]
    waug = np.concatenate([g0 * wb, np.ones((CP, 1), np.float32),
                           np.zeros((CP, 32 - H - 1), np.float32)], 1).astype(BF)
    c1n = -(g0 * wb).sum(0)[None, :].astype(np.float32)
    c2 = (np.asarray(ln0_b, np.float32) @ wb)[None, :].astype(np.float32)
    wa = np.asarray(Wa, np.float32).astype(BF)

    A = np.asarray(A_I, np.float32)[0]
    Z = np.asarray(Z_II)[0]
    Beta = np.asarray(Beta_II, np.float32)[0]

    shared = dict(a_full=A, wq=wq, wk=wk, wv=wv, wg=wg, wa=wa, waug=waug,
                  bq=bq, bk=bk, bv=bv, bg=bg, c1n=c1n, c2=c2)
    in_maps = []
    for cid in range(ncores):
        sl = slice(cid * ls, (cid + 1) * ls)
        zt = np.ascontiguousarray(
            Z[sl].transpose(0, 2, 1)).astype(BF)  # [ls, 128, 768]
        m = dict(shared)
        m.update(zt=zt, beta=Beta[sl].astype(BF), a_i=A[sl])
        in_maps.append(m)
    return in_maps


_NC_CACHE = {}


def get_nc(ls=LS):
    if ls not in _NC_CACHE:
        _NC_CACHE[ls] = build_program(ls)
    return _NC_CACHE[ls]


def kernel(**inputs):
    from concourse.bass_utils import run_bass_kernel_spmd

    nc = get_nc()
    in_maps = host_prep(**inputs)
    res = run_bass_kernel_spmd(nc, in_maps, core_ids=list(range(NCORES)))
    out = np.concatenate([np.asarray(r["out"]) for r in res.results], axis=0)
    return out.reshape(1, L, CA).astype(np.float32)
